# revision 1
# baseline (speedup 1.0000x reference)
"""Trainium2 Bass kernel for nn_Attention_b (tanh-attention with masked_scatter).

Data-parallel over batch: each of 8 NeuronCores owns 4 batches. Per core:
  phase 1  z = W1 @ h_i + (W2 @ h_t + b)   (fp32r GEMM, [A, rows])
           m = tanh(z); y = u . m          (raw scores, [rows])
  comm     AllGather of the per-chunk score slice across the 8 cores
  phase 2  masked_scatter selection (0/1 matrix against gathered scores)
           + online softmax over the sequence (flash-style, chunked)
  phase 3  s_acc += e * h_i  (fused DVE multiply-reduce on the resident
           h_i chunk -- h_i is read from HBM exactly once)
"""
import sys

for _p in ("/opt/trn_rl_repo",):
    if _p not in sys.path:
        sys.path.insert(0, _p)

import numpy as np

import concourse.bacc as bacc
import concourse.tile as tile
from concourse import mybir
from concourse.bass_utils import run_bass_kernel_spmd
from concourse.dve_ops import TENSOR_TENSOR_REDUCE
from concourse.masks import make_identity

NCORES = 8
B, S, H, A = 32, 2048, 1024, 256
BL = B // NCORES          # local batches per core
NEG = np.float32(-1e20)

f32 = mybir.dt.float32
f32r = mybir.dt.float32r


def build_kernel(S=S, H=H, A=A, C=256, hi_bufs=4, clist=None):
    KT = H // 128             # contraction tiles
    AT = A // 128             # score tiles
    if clist is None:
        clist = [C] * (S // C)
    offs = np.concatenate([[0], np.cumsum(clist)]).tolist()
    NCH = len(clist)
    assert offs[-1] == S and H % 128 == 0 and A % 128 == 0

    nc = bacc.Bacc("TRN2", target_bir_lowering=False, debug=False,
                   num_devices=NCORES)

    # big operands are declared float32r (same bits as f32) so the plain
    # HWDGE DMA path can be used -- no SWDGE cast, no Q7 descriptor work
    hi5 = nc.declare_dram_parameter("hi5", [128, KT * BL * S], f32r,
                                    isOutput=False)
    w1t = nc.declare_dram_parameter("w1t", [H, A], f32r, isOutput=False)
    cb2 = nc.declare_dram_parameter("cb2", [128, AT, BL], f32, isOutput=False)
    u2 = nc.declare_dram_parameter("u2", [128, AT], f32r, isOutput=False)
    sel = nc.declare_dram_parameter("sel", [B + 1, BL, S], f32,
                                    isOutput=False)
    out = nc.declare_dram_parameter("out", [BL, H], f32, isOutput=True)

    with tile.TileContext(nc) as tc:
        with (
            tc.tile_pool(name="consts", bufs=1) as cp,
            tc.tile_pool(name="hi", bufs=hi_bufs) as hip,
            tc.tile_pool(name="m", bufs=2) as mp,
            tc.tile_pool(name="small", bufs=3) as sp,
            tc.tile_pool(name="ebc", bufs=2) as ebp,
            tc.tile_pool(name="sacc", bufs=2) as sap,
            tc.tile_pool(name="pz", bufs=2, space="PSUM") as pz,
            tc.tile_pool(name="py", bufs=2, space="PSUM") as py,
            tc.tile_pool(name="dram", bufs=NCH, space="DRAM") as dp,
        ):
            # ---- preload replicated constants
            w1_sb = cp.tile([128, KT, A], f32r)
            nc.sync.dma_start(
                out=w1_sb, in_=w1t.rearrange("(t p) a -> p t a", p=128))
            u_sb = cp.tile([128, AT], f32r)
            nc.sync.dma_start(out=u_sb, in_=u2[:, :])
            cb_sb = cp.tile([128, AT, BL], f32)
            nc.sync.dma_start(out=cb_sb, in_=cb2[:, :, :])
            ident = cp.tile([128, 128], f32)
            make_identity(nc, ident)
            ones_sb = cp.tile([B + 1, 1], f32r)
            nc.vector.memset(ones_sb.bitcast(f32), 1.0)

            # ---- per-chunk softmax stats (combined once at the end)
            mall = cp.tile([1, BL, NCH], f32)
            lall = cp.tile([1, BL, NCH], f32)
            saccs = [cp.tile([128, KT, BL], f32, name=f"sacc{i}")
                     for i in range(NCH)]


            carries = []

            def phase1(i):
                Ci, off = clist[i], offs[i]
                hi_sb = hip.tile([128, KT, BL, Ci], f32r, tag="hi")
                nc.sync.dma_start(
                    out=hi_sb.rearrange("p t b s -> p (t b s)"),
                    in_=hi5[:, KT * BL * off : KT * BL * (off + Ci)])
                sel_c = sp.tile([B + 1, BL, Ci], f32, tag="selc")
                nc.scalar.dma_start(out=sel_c, in_=sel[:, :, off : off + Ci])
                m_r = mp.tile([128, AT, BL, Ci], f32r, tag="m")
                for at in range(AT):
                    z_ps = pz.tile([128, BL, Ci], f32, tag="z")
                    for r in range(BL // 2):
                        for kt in range(KT):
                            nc.tensor.matmul(
                                z_ps[:, 2 * r : 2 * r + 2, :],
                                w1_sb[:, kt, at * 128 : (at + 1) * 128],
                                hi_sb[:, kt, 2 * r : 2 * r + 2, :],
                                start=(kt == 0), stop=(kt == KT - 1),
                            )
                    for b in range(BL):
                        nc.scalar.activation(
                            out=m_r[:, at, b, :], in_=z_ps[:, b, :],
                            func=mybir.ActivationFunctionType.Tanh,
                            bias=cb_sb[:, at, b : b + 1], scale=1.0,
                        )
                y_ps = py.tile([1, BL, Ci], f32, tag="y")
                for r in range(BL // 2):
                    for at in range(AT):
                        nc.tensor.matmul(
                            y_ps[:, 2 * r : 2 * r + 2, :],
                            u_sb[:, at : at + 1],
                            m_r[:, at, 2 * r : 2 * r + 2, :],
                            start=(at == 0), stop=(at == AT - 1),
                        )
                y_sb = sp.tile([1, BL, Ci], f32, tag="ysb", bufs=2)
                nc.scalar.activation(out=y_sb, in_=y_ps,
                                     func=mybir.ActivationFunctionType.Copy)

                ag_in = dp.tile([BL * Ci], f32, tag="agin")
                nc.scalar.dma_start(
                    out=ag_in.rearrange("(o n) -> o n", o=1),
                    in_=y_sb.rearrange("p b s -> p (b s)"))
                ag_out = dp.tile([B * Ci], f32, tag="agout",
                                 addr_space="Shared")
                nc.gpsimd.collective_compute(
                    "AllGather", mybir.AluOpType.bypass,
                    ins=[ag_in[:]], outs=[ag_out[:]],
                    replica_groups=[list(range(NCORES))],
                )
                y32 = sp.tile([B + 1, Ci], f32, tag="y32")
                nc.gpsimd.memset(y32[B : B + 1, :], 1.0)
                nc.scalar.dma_start(
                    out=y32[:B, :], in_=ag_out.rearrange("(j s) -> j s", s=Ci))
                return dict(hi_sb=hi_sb, sel_c=sel_c, y32=y32, i=i, Ci=Ci)

            def phase2(c):
                i, Ci = c["i"], c["Ci"]
                sel_c, y32 = c["sel_c"], c["y32"]
                # masked_scatter selection: one-hot rows (plus a -1e20 mask
                # row) dotted with [y; 1]
                bt_ps = py.tile([1, BL, Ci], f32, tag="y")
                selY = sp.tile([B + 1, BL, Ci], f32r, tag="selY", bufs=2)
                nc.vector.tensor_mul(
                    selY, sel_c,
                    y32.rearrange("j (o s) -> j o s", o=1)
                       .broadcast_to([B + 1, BL, Ci]))
                for hf in range(2):
                    nc.tensor.matmul(
                        bt_ps[:, 2 * hf : 2 * hf + 2, :], ones_sb,
                        selY[:, 2 * hf : 2 * hf + 2, :],
                        start=True, stop=True)

                # chunk-local max -> no cross-chunk recurrence
                cmax = sp.tile([1, BL], f32, tag="cmax")
                nc.vector.tensor_reduce(
                    out=cmax.rearrange("p (b o) -> p b o", o=1), in_=bt_ps,
                    axis=mybir.AxisListType.X, op=mybir.AluOpType.max)
                nc.vector.tensor_copy(mall[:, :, i], cmax)
                nmnew = sp.tile([1, BL], f32, tag="nmnew")
                nc.vector.tensor_scalar_mul(nmnew, cmax, -1.0)
                e4 = sp.tile([1, BL, Ci], f32, tag="e4", bufs=2)
                for b in range(BL):
                    nc.scalar.activation(
                        out=e4[:, b, :], in_=bt_ps[:, b, :],
                        func=mybir.ActivationFunctionType.Exp,
                        bias=nmnew[:, b : b + 1], scale=1.0,
                        accum_out=lall[:, b, i : i + 1])
                e_bc = ebp.tile([128, BL, Ci], f32, tag="ebc")
                nc.gpsimd.partition_broadcast(
                    e_bc.rearrange("p b s -> p (b s)"),
                    e4.rearrange("p b s -> p (b s)"))
                c["ebc"] = e_bc

            def phase3(c):
                i, Ci = c["i"], c["Ci"]
                sacc_i = saccs[i]
                ttr_scr = sp.tile([128, 1], f32, tag="ttrscr")
                hi_sb = c["hi_sb"]
                e_bc_all = c["ebc"]
                for b in range(BL):
                    e_bc = e_bc_all[:, b, :]
                    for kt in range(KT):
                        nc.vector._custom_dve(
                            TENSOR_TENSOR_REDUCE,
                            out=ttr_scr.broadcast_to([128, Ci]),
                            in0=hi_sb[:, kt, b, :].bitcast(f32),
                            in1=e_bc,
                            s0=0.0, s1=1.0,
                            accum_out=sacc_i[:, kt, b : b + 1],
                        )

            for i in range(NCH):
                carries.append(phase1(i))
                if len(carries) >= 2:
                    phase2(carries[-2])
                if len(carries) >= 3:
                    phase3(carries.pop(0))
            phase2(carries[-1])
            while carries:
                phase3(carries.pop(0))

            # ---- finalize: combine chunk partials, divide, transpose, store
            M = sp.tile([1, BL], f32, tag="cmax")
            nc.vector.tensor_reduce(
                out=M.rearrange("p (b o) -> p b o", o=1), in_=mall,
                axis=mybir.AxisListType.X, op=mybir.AluOpType.max)
            nM = sp.tile([1, BL], f32, tag="nmnew")
            nc.vector.tensor_scalar_mul(nM, M, -1.0)
            w = sp.tile([1, BL, NCH], f32, tag="w")
            for b in range(BL):
                nc.scalar.activation(
                    out=w[:, b, :], in_=mall[:, b, :],
                    func=mybir.ActivationFunctionType.Exp,
                    bias=nM[:, b : b + 1], scale=1.0)
            wl = sp.tile([1, BL, NCH], f32, tag="wl")
            nc.vector.tensor_mul(wl, w, lall)
            lsum = sp.tile([1, BL], f32, tag="lsum")
            nc.vector.tensor_reduce(
                out=lsum.rearrange("p (b o) -> p b o", o=1), in_=wl,
                axis=mybir.AxisListType.X, op=mybir.AluOpType.add)
            il = sp.tile([1, BL], f32, tag="il")
            nc.vector.reciprocal(il, lsum)
            wn = sp.tile([1, BL, NCH], f32, tag="wn")
            for b in range(BL):
                nc.vector.tensor_scalar_mul(wn[:, b, :], w[:, b, :],
                                            il[:, b : b + 1])
            wbc = ebp.tile([128, BL, NCH], f32, tag="wbc")
            nc.gpsimd.partition_broadcast(
                wbc.rearrange("p b n -> p (b n)"),
                wn.rearrange("p b n -> p (b n)"))
            sfin = sap.tile([128, KT, BL], f32, tag="sacc")
            for i in range(NCH):
                for b in range(BL):
                    if i == 0:
                        nc.vector.tensor_scalar_mul(
                            sfin[:, :, b], saccs[0][:, :, b],
                            wbc[:, b, 0:1])
                    else:
                        tmp = sp.tile([128, KT], f32, tag="ftmp")
                        nc.vector.tensor_scalar_mul(
                            tmp, saccs[i][:, :, b], wbc[:, b, i : i + 1])
                        nc.vector.tensor_add(
                            sfin[:, :, b], sfin[:, :, b], tmp)
            t_ps = py.tile([KT * BL, 128], f32, tag="y")
            nc.tensor.transpose(
                t_ps, sfin.rearrange("p t b -> p (t b)"), ident)
            t_sb = sp.tile([KT * BL, 128], f32, tag="tsb")
            nc.vector.tensor_copy(t_sb, t_ps)
            for t in range(KT):
                nc.sync.dma_start(
                    out=out[:, t * 128 : (t + 1) * 128],
                    in_=t_sb[t * BL : (t + 1) * BL, :])

    nc.compile()
    _split_pe_waits(nc)
    return nc


def _split_pe_waits(nc):
    """TRN2 PE instructions (S3_LW encoding) take a single sync-wait slot.
    Bacc's legalization misses some Matmults; hoist excess waits onto
    dedicated PE NoOps inserted directly before the offender."""
    for f in nc.m.functions:
        for bb in f.blocks:
            insts = bb.instructions
            i = 0
            while i < len(insts):
                ins = insts[i]
                if type(ins).__name__ in ("InstMatmult", "InstNoOp") and \
                        ins.engine == mybir.EngineType.PE:
                    si = ins.sync_info
                    if si is not None and len(si.on_wait) > 1:
                        extra, keep = si.on_wait[:-1], si.on_wait[-1:]
                        for w in extra:
                            nop = mybir.InstNoOp(
                                name=nc.get_next_instruction_name(),
                                ins=[], outs=[])
                            nop.engine = ins.engine
                            nop.sync_info = mybir.SyncInfo(
                                on_wait=[w], on_update=[])
                            nc.register_instruction(nop)
                            insts.insert(i, nop)
                            i += 1
                        si.on_wait = keep
                i += 1


def prep_inputs(h_i, h_t, mask, W, b, u, S=S, H=H, A=A, C=256, clist=None):
    """Shard + lay out the full inputs for the 8 cores."""
    h_i = np.asarray(h_i, np.float32)
    h_t = np.asarray(h_t, np.float32)
    mask = np.asarray(mask, bool)
    W = np.asarray(W, np.float32)
    b = np.asarray(b, np.float32)
    u = np.asarray(u, np.float32)

    KT = H // 128
    AT = A // 128
    if clist is None:
        clist = [C] * (S // C)
    offs = np.concatenate([[0], np.cumsum(clist)]).astype(int)
    w1t = np.ascontiguousarray(W[:, :H].T)                      # [H, A]
    cb = h_t @ W[:, H:].T + b                                   # [B, A]
    cb2s = np.ascontiguousarray(
        cb.reshape(B, AT, 128).transpose(2, 1, 0))              # [128, AT, B]
    u2 = np.ascontiguousarray(u[:, 0].reshape(AT, 128).T)       # [128, AT]

    pos = np.clip(np.cumsum(mask.astype(np.int64), axis=0) - 1, 0, None)
    onehot = (np.arange(B)[None, :, None] == pos[:, None, :]) & mask[:, None, :]
    selall = onehot.astype(np.float32)                          # [B, B, S]
    negall = np.where(mask, np.float32(0), NEG).astype(np.float32)  # [B, S]
    sel33 = np.concatenate([selall, negall[:, None, :]], axis=1)  # [B, B+1, S]

    in_maps = []
    for c in range(NCORES):
        bs = slice(c * BL, (c + 1) * BL)
        # hi5[p, block_i ++ (t, b, s)] = h_i[b, off_i+s, t*128+p]
        hcf = h_i[bs].reshape(BL, S, KT, 128)
        blocks = []
        for ci, off in zip(clist, offs[:-1]):
            hc = hcf[:, off : off + ci]                     # [BL, ci, KT, 128]
            blocks.append(hc.transpose(3, 2, 0, 1).reshape(128, KT * BL * ci))
        hi5 = np.ascontiguousarray(np.concatenate(blocks, axis=1))
        in_maps.append({
            "hi5": hi5,
            "w1t": w1t,
            "cb2": np.ascontiguousarray(cb2s[:, :, bs]),
            "u2": u2,
            "sel": np.ascontiguousarray(sel33[bs].transpose(1, 0, 2)),
        })
    return in_maps


_NC_CACHE = {}


CLIST = [128, 128] + [256] * 7


def _get_nc():
    if "nc" not in _NC_CACHE:
        _NC_CACHE["nc"] = build_kernel(clist=CLIST)
    return _NC_CACHE["nc"]


def kernel(h_i, h_t, mask, W, b, u):
    nc = _get_nc()
    in_maps = prep_inputs(h_i, h_t, mask, W, b, u, clist=CLIST)
    res = run_bass_kernel_spmd(nc, in_maps, list(range(NCORES)))
    return np.concatenate([res.results[c]["out"] for c in range(NCORES)],
                          axis=0)



# revision 21
# speedup vs baseline: 1.3286x; 1.3286x over previous
"""Trainium2 Bass kernel for nn_Attention_b (tanh-attention with masked_scatter).

Data-parallel over batch: each of 8 NeuronCores owns 4 batches. Per core:
  sweep A  z = W1 @ h_i + (W2 @ h_t + b)   (fp16 GEMM, [A, rows])
           m = tanh(z); y = u . m          (raw scores, [rows])
           AllGather of score pairs across the 8 cores
           masked_scatter selection (0/1 matrix against gathered scores)
           chunk-local max + exp + running sums (flash-style)
  sweep B  combine chunk stats -> per-chunk weights wn = exp(m_i - M)/L
           eT[s, b] = e_all * wn via rank-1 PE matmuls (transpose for free)
           s[b, :] = sum_s eT * h_iT on the PE (contraction over s on
           partitions), accumulating all chunks into one PSUM tile.
h_i is sent twice (fp16): once h-major for the GEMM, once s-major for the
weighted sum, so no engine ever transposes the big tensor on chip.
"""
import sys

for _p in ("/opt/trn_rl_repo",):
    if _p not in sys.path:
        sys.path.insert(0, _p)

import numpy as np

import concourse.bacc as bacc
import concourse.tile as tile
from concourse import mybir
from concourse.bass_utils import run_bass_kernel_spmd
from concourse.masks import make_identity

NCORES = 8
B, S, H, A = 32, 2048, 1024, 256
BL = B // NCORES          # local batches per core
NEG = np.float32(-1e20)

f32 = mybir.dt.float32
f32r = mybir.dt.float32r
f16 = mybir.dt.float16


def build_kernel(S=S, H=H, A=A, C=256, hi_bufs=3, hit_bufs=4, debug_out=False):
    KT = H // 128             # contraction tiles
    AT = A // 128             # score tiles
    NCH = S // C              # chunks (phase-1/2 granularity)
    NPR = NCH // 2            # AllGather pairs
    NSB = S // 128            # seq 128-blocks (phase-3 granularity)
    SBC = C // 128            # seq blocks per chunk
    assert H % 128 == 0 and A % 128 == 0 and S % (2 * C) == 0

    nc = bacc.Bacc("TRN2", target_bir_lowering=False, debug=False,
                   num_devices=NCORES)

    hi5 = nc.declare_dram_parameter("hi5", [128, KT * BL * S], f16,
                                    isOutput=False)
    hit = nc.declare_dram_parameter("hit", [128, NSB * BL * H], f16,
                                    isOutput=False)
    w1t = nc.declare_dram_parameter("w1t", [H, A], f16, isOutput=False)
    cb2 = nc.declare_dram_parameter("cb2", [128, AT, BL], f32, isOutput=False)
    u2 = nc.declare_dram_parameter("u2", [128, AT], f16, isOutput=False)
    sel = nc.declare_dram_parameter("sel", [B + 1, BL, S], f32,
                                    isOutput=False)
    out = nc.declare_dram_parameter("out", [BL, H], f32, isOutput=True)
    if debug_out:
        dbg_y = nc.declare_dram_parameter("dbg_y", [B + 1, 2, C], f32,
                                          isOutput=True)
        dbg_e = nc.declare_dram_parameter("dbg_e", [1, BL, S], f32,
                                          isOutput=True)
        dbg_st = nc.declare_dram_parameter("dbg_st", [1, BL, 4 * (S // C)],
                                           f32, isOutput=True)
        dbg_eT = nc.declare_dram_parameter("dbg_eT", [128, (S // 128) * BL],
                                           f16, isOutput=True)
        dbg_m = nc.declare_dram_parameter("dbg_m", [128, (A // 128) * BL * C],
                                          f16, isOutput=True)
        dbg_ys = nc.declare_dram_parameter("dbg_ys", [1, 2 * BL * C], f32,
                                           isOutput=True)
        dbg_sf = nc.declare_dram_parameter("dbg_sf", [128, (H // 128) * BL],
                                           f32, isOutput=True)

    with tile.TileContext(nc) as tc:
        with (
            tc.tile_pool(name="consts", bufs=1) as cp,
            tc.tile_pool(name="hi", bufs=hi_bufs) as hip,
            tc.tile_pool(name="hit", bufs=hit_bufs) as htp,
            tc.tile_pool(name="m", bufs=2) as mp,
            tc.tile_pool(name="small", bufs=3) as sp,
            tc.tile_pool(name="pz", bufs=2, space="PSUM") as pz,
            tc.tile_pool(name="py", bufs=2, space="PSUM") as py,
            tc.tile_pool(name="dram", bufs=2 * NPR, space="DRAM") as dp,
        ):
            # ---- preload replicated constants
            w1_sb = cp.tile([128, KT, A], f16)
            nc.sync.dma_start(
                out=w1_sb, in_=w1t.rearrange("(t p) a -> p t a", p=128))
            u_sb = cp.tile([128, AT], f16)
            nc.sync.dma_start(out=u_sb, in_=u2[:, :])
            cb_sb = cp.tile([128, AT, BL], f32)
            nc.sync.dma_start(out=cb_sb, in_=cb2[:, :, :])
            ones_sb = cp.tile([B + 1, 1], f32r)
            nc.vector.memset(ones_sb.bitcast(f32), 1.0)
            ident = cp.tile([128, 128], f32)
            make_identity(nc, ident)

            # ---- per-chunk softmax stats + resident exp values
            mall = cp.tile([1, BL, NCH], f32)
            lall = cp.tile([1, BL, NCH], f32)
            e_all = cp.tile([1, BL, S], f32)
            eT = cp.tile([128, NSB, BL], f16)

            def phase1(i):
                off = C * i
                hi_sb = hip.tile([128, KT, BL, C], f16, tag="hi")
                nc.sync.dma_start(
                    out=hi_sb.rearrange("p t b s -> p (t b s)"),
                    in_=hi5[:, KT * BL * off : KT * BL * (off + C)])
                sel_c = sp.tile([B + 1, BL, C], f32, tag="selc", bufs=4)
                nc.scalar.dma_start(out=sel_c, in_=sel[:, :, off : off + C])
                m_r = mp.tile([128, AT, BL, C], f16, tag="m")
                for at in range(AT):
                    z_ps = pz.tile([128, BL, C], f32, tag="z")
                    for r in range(BL // 2):
                        for kt in range(KT):
                            nc.tensor.matmul(
                                z_ps[:, 2 * r : 2 * r + 2, :],
                                w1_sb[:, kt, at * 128 : (at + 1) * 128],
                                hi_sb[:, kt, 2 * r : 2 * r + 2, :],
                                start=(kt == 0), stop=(kt == KT - 1),
                            )
                    for b in range(BL):
                        nc.scalar.activation(
                            out=m_r[:, at, b, :], in_=z_ps[:, b, :],
                            func=mybir.ActivationFunctionType.Tanh,
                            bias=cb_sb[:, at, b : b + 1], scale=1.0,
                        )
                y_ps = py.tile([1, BL, C], f32, tag="y")
                for r in range(BL // 2):
                    for at in range(AT):
                        nc.tensor.matmul(
                            y_ps[:, 2 * r : 2 * r + 2, :],
                            u_sb[:, at : at + 1],
                            m_r[:, at, 2 * r : 2 * r + 2, :],
                            start=(at == 0), stop=(at == AT - 1),
                        )
                if debug_out and i == 0:
                    nc.scalar.dma_start(
                        out=dbg_m[:, :],
                        in_=m_r.rearrange("p a b s -> p (a b s)"))
                return dict(y_ps=y_ps, sel_c=sel_c, i=i)

            def aggather(c0, c1):
                """Copy the two chunks' scores out and AllGather the pair."""
                y_sb = sp.tile([1, BL, 2, C], f32, tag="ysb", bufs=2)
                nc.scalar.activation(out=y_sb[:, :, 0, :], in_=c0["y_ps"],
                                     func=mybir.ActivationFunctionType.Copy)
                nc.scalar.activation(out=y_sb[:, :, 1, :], in_=c1["y_ps"],
                                     func=mybir.ActivationFunctionType.Copy)
                ag_in = dp.tile([2 * BL * C], f32, tag="agin")
                nc.scalar.dma_start(
                    out=ag_in.rearrange("(o n) -> o n", o=1),
                    in_=y_sb.rearrange("p b c s -> p (b c s)"))
                if debug_out and c0["i"] == 0:
                    nc.scalar.dma_start(
                        out=dbg_ys[:, :],
                        in_=y_sb.rearrange("p b c s -> p (b c s)"))
                ag_out = dp.tile([2 * B * C], f32, tag="agout",
                                 addr_space="Shared")
                nc.gpsimd.collective_compute(
                    "AllGather", mybir.AluOpType.bypass,
                    ins=[ag_in[:]], outs=[ag_out[:]],
                    replica_groups=[list(range(NCORES))],
                )
                # gathered rows: [(core, b), (chunk-of-pair, s)]
                y32 = sp.tile([B + 1, 2, C], f32, tag="y32")
                nc.gpsimd.memset(y32[B : B + 1, :, :], 1.0)
                nc.scalar.dma_start(
                    out=y32[:B].rearrange("j c s -> j (c s)"),
                    in_=ag_out.rearrange("(j n) -> j n", n=2 * C))
                if debug_out and c0["i"] == 0:
                    nc.scalar.dma_start(out=dbg_y[:, :, :], in_=y32)
                return dict(y32=y32, h0=c0, h1=c1)

            def phase2(c, y32):
                i, sel_c = c["i"], c["sel_c"]
                # masked_scatter selection: one-hot rows (plus a -1e20 mask
                # row) dotted with [y; 1]
                bt_ps = py.tile([1, BL, C], f32, tag="y")
                selY = sp.tile([B + 1, BL, C], f32r, tag="selY", bufs=2)
                nc.vector.tensor_mul(
                    selY, sel_c,
                    y32.rearrange("j (o s) -> j o s", o=1)
                       .broadcast_to([B + 1, BL, C]))
                for hf in range(2):
                    nc.tensor.matmul(
                        bt_ps[:, 2 * hf : 2 * hf + 2, :], ones_sb,
                        selY[:, 2 * hf : 2 * hf + 2, :],
                        start=True, stop=True)

                # chunk-local max -> no cross-chunk recurrence
                cmax = sp.tile([1, BL], f32, tag="cmax")
                nc.vector.tensor_reduce(
                    out=cmax.rearrange("p (b o) -> p b o", o=1), in_=bt_ps,
                    axis=mybir.AxisListType.X, op=mybir.AluOpType.max)
                nc.vector.tensor_copy(mall[:, :, i], cmax)
                nmnew = sp.tile([1, BL], f32, tag="nmnew")
                nc.vector.tensor_scalar_mul(nmnew, cmax, -1.0)
                for b in range(BL):
                    nc.scalar.activation(
                        out=e_all[:, b, C * i : C * (i + 1)],
                        in_=bt_ps[:, b, :],
                        func=mybir.ActivationFunctionType.Exp,
                        bias=nmnew[:, b : b + 1], scale=1.0,
                        accum_out=lall[:, b, i : i + 1])

            # ---- sweep A: scores + AllGather + softmax stats, pipelined
            pairs = []
            for p in range(NPR):
                c0 = phase1(2 * p)
                c1 = phase1(2 * p + 1)
                pairs.append(aggather(c0, c1))
                if p >= 1:
                    ag = pairs[p - 1]
                    phase2(ag["h0"], ag["y32"][:, 0, :])
                    phase2(ag["h1"], ag["y32"][:, 1, :])
            ag = pairs[-1]
            phase2(ag["h0"], ag["y32"][:, 0, :])
            phase2(ag["h1"], ag["y32"][:, 1, :])

            # ---- start streaming the s-major copy of h_i (phase-3 operand)
            hit_tiles = []
            for j in range(NCH):
                ht = htp.tile([128, SBC, BL, H], f16, tag="hit")
                nc.sync.dma_start(
                    out=ht.rearrange("p c b h -> p (c b h)"),
                    in_=hit[:, SBC * BL * H * j : SBC * BL * H * (j + 1)])
                hit_tiles.append(ht)

            # ---- combine chunk stats: wn[b, i] = exp(m_i - M) / L_b
            M = sp.tile([1, BL], f32, tag="cmax")
            nc.vector.tensor_reduce(
                out=M.rearrange("p (b o) -> p b o", o=1), in_=mall,
                axis=mybir.AxisListType.X, op=mybir.AluOpType.max)
            nM = sp.tile([1, BL], f32, tag="nmnew")
            nc.vector.tensor_scalar_mul(nM, M, -1.0)
            w = sp.tile([1, BL, NCH], f32, tag="w")
            for b in range(BL):
                nc.scalar.activation(
                    out=w[:, b, :], in_=mall[:, b, :],
                    func=mybir.ActivationFunctionType.Exp,
                    bias=nM[:, b : b + 1], scale=1.0)
            wl = sp.tile([1, BL, NCH], f32, tag="wl")
            nc.vector.tensor_mul(wl, w, lall)
            lsum = sp.tile([1, BL], f32, tag="lsum")
            nc.vector.tensor_reduce(
                out=lsum.rearrange("p (b o) -> p b o", o=1), in_=wl,
                axis=mybir.AxisListType.X, op=mybir.AluOpType.add)
            il = sp.tile([1, BL], f32, tag="il")
            nc.vector.reciprocal(il, lsum)
            wn = sp.tile([1, BL, NCH], f32, tag="wn")
            for b in range(BL):
                nc.vector.tensor_scalar_mul(wn[:, b, :], w[:, b, :],
                                            il[:, b : b + 1])

            # ---- eT[s, (sb, b)] = e_all[b, s] * wn[b, chunk(s)], transposed
            # onto s-partitions via rank-1 matmuls (k=1)
            eT_ps = py.tile([128, NSB, BL], f32, tag="y")
            for sb in range(NSB):
                for b in range(BL):
                    nc.tensor.matmul(
                        eT_ps[:, sb, b : b + 1],
                        e_all[:, b, sb * 128 : (sb + 1) * 128],
                        wn[:, b, sb // SBC : sb // SBC + 1],
                        start=True, stop=True)
            nc.scalar.activation(out=eT, in_=eT_ps,
                                 func=mybir.ActivationFunctionType.Copy)
            if debug_out:
                nc.scalar.dma_start(out=dbg_e[:, :, :], in_=e_all)
                st = sp.tile([1, BL, 4 * NCH], f32, tag="dbgst")
                nc.vector.tensor_copy(st[:, :, 0 * NCH : 1 * NCH], mall)
                nc.vector.tensor_copy(st[:, :, 1 * NCH : 2 * NCH], lall)
                nc.vector.tensor_copy(st[:, :, 2 * NCH : 3 * NCH], w)
                nc.vector.tensor_copy(st[:, :, 3 * NCH : 4 * NCH], wn)
                nc.scalar.dma_start(out=dbg_st[:, :, :], in_=st)
                nc.scalar.dma_start(
                    out=dbg_eT[:, :], in_=eT.rearrange("p n b -> p (n b)"))

            # ---- sweep B: s[h, b] += sum_s hiT[s, h] * eT[s, b] on the PE
            # (hiT block stationary, eT moving; PSUM-accumulated over s)
            # all-start=False + explicit zero: a start=True would mark the
            # whole 2KB PSUM zero region pending and wipe the other
            # interleaved groups' first contributions
            p3a = py.tile([128, KT, BL], f32, tag="y")
            nc.vector.memset(p3a, 0.0)
            for j in range(NCH):
                ht = hit_tiles[j]
                for c in range(SBC):
                    sb = SBC * j + c
                    for b in range(BL):
                        for hb in range(KT):
                            nc.tensor.matmul(
                                p3a[:, hb, b : b + 1],
                                ht[:, c, b, 128 * hb : 128 * (hb + 1)],
                                eT[:, sb, b : b + 1],
                                start=False, stop=(sb == NSB - 1),
                                skip_group_check=True,
                            )
            # transpose [h, (kt, b)] -> [(kt, b), h] and store
            sfin = sp.tile([128, KT, BL], f32, tag="sfin")
            nc.scalar.activation(out=sfin, in_=p3a,
                                 func=mybir.ActivationFunctionType.Copy)
            if debug_out:
                nc.scalar.dma_start(
                    out=dbg_sf[:, :],
                    in_=sfin.rearrange("p t b -> p (t b)"))
            t_ps = py.tile([KT * BL, 128], f32, tag="y")
            nc.tensor.transpose(
                t_ps, sfin.rearrange("p t b -> p (t b)"), ident)
            t_sb = sp.tile([KT * BL, 128], f32, tag="tsb")
            nc.vector.tensor_copy(t_sb, t_ps)
            for t in range(KT):
                nc.sync.dma_start(
                    out=out[:, t * 128 : (t + 1) * 128],
                    in_=t_sb[t * BL : (t + 1) * BL, :])

    nc.compile()
    _split_pe_waits(nc)
    return nc


def _split_pe_waits(nc):
    """TRN2 PE instructions (S3_LW encoding) take a single sync-wait slot.
    Bacc's legalization misses some Matmults; hoist excess waits onto
    dedicated PE NoOps inserted directly before the offender."""
    for f in nc.m.functions:
        for bb in f.blocks:
            insts = bb.instructions
            i = 0
            while i < len(insts):
                ins = insts[i]
                if type(ins).__name__ in ("InstMatmult", "InstNoOp") and \
                        ins.engine == mybir.EngineType.PE:
                    si = ins.sync_info
                    if si is not None and len(si.on_wait) > 1:
                        extra, keep = si.on_wait[:-1], si.on_wait[-1:]
                        for w in extra:
                            nop = mybir.InstNoOp(
                                name=nc.get_next_instruction_name(),
                                ins=[], outs=[])
                            nop.engine = ins.engine
                            nop.sync_info = mybir.SyncInfo(
                                on_wait=[w], on_update=[])
                            nc.register_instruction(nop)
                            insts.insert(i, nop)
                            i += 1
                        si.on_wait = keep
                i += 1


def prep_inputs(h_i, h_t, mask, W, b, u, S=S, H=H, A=A, C=256):
    """Shard + lay out the full inputs for the 8 cores."""
    h_i = np.asarray(h_i, np.float32)
    h_t = np.asarray(h_t, np.float32)
    mask = np.asarray(mask, bool)
    W = np.asarray(W, np.float32)
    b = np.asarray(b, np.float32)
    u = np.asarray(u, np.float32)

    KT = H // 128
    AT = A // 128
    NSB = S // 128
    w1t = np.ascontiguousarray(W[:, :H].T).astype(np.float16)   # [H, A]
    cb = h_t @ W[:, H:].T + b                                   # [B, A]
    cb2s = np.ascontiguousarray(
        cb.reshape(B, AT, 128).transpose(2, 1, 0))              # [128, AT, B]
    u2 = np.ascontiguousarray(
        u[:, 0].reshape(AT, 128).T).astype(np.float16)          # [128, AT]

    pos = np.clip(np.cumsum(mask.astype(np.int64), axis=0) - 1, 0, None)
    onehot = (np.arange(B)[None, :, None] == pos[:, None, :]) & mask[:, None, :]
    selall = onehot.astype(np.float32)                          # [B, B, S]
    negall = np.where(mask, np.float32(0), NEG).astype(np.float32)  # [B, S]
    sel33 = np.concatenate([selall, negall[:, None, :]], axis=1)  # [B, B+1, S]

    in_maps = []
    for c in range(NCORES):
        bs = slice(c * BL, (c + 1) * BL)
        hcf = h_i[bs].astype(np.float16)                    # [BL, S, H]
        # hi5[p, chunk_i ++ (t, b, s)] = h_i[b, off_i+s, t*128+p]
        h4 = hcf.reshape(BL, S // C, C, KT, 128)
        hi5 = np.ascontiguousarray(
            h4.transpose(4, 1, 3, 0, 2).reshape(128, KT * BL * S))
        # hit[p, (sb, b, h)] = h_i[b, sb*128+p, h]
        h5 = hcf.reshape(BL, NSB, 128, H)
        hitm = np.ascontiguousarray(
            h5.transpose(2, 1, 0, 3).reshape(128, NSB * BL * H))
        in_maps.append({
            "hi5": hi5,
            "hit": hitm,
            "w1t": w1t,
            "cb2": np.ascontiguousarray(cb2s[:, :, bs]),
            "u2": u2,
            "sel": np.ascontiguousarray(sel33[bs].transpose(1, 0, 2)),
        })
    return in_maps


_NC_CACHE = {}


def _get_nc():
    if "nc" not in _NC_CACHE:
        _NC_CACHE["nc"] = build_kernel()
    return _NC_CACHE["nc"]


def kernel(h_i, h_t, mask, W, b, u):
    nc = _get_nc()
    in_maps = prep_inputs(h_i, h_t, mask, W, b, u)
    res = run_bass_kernel_spmd(nc, in_maps, list(range(NCORES)))
    return np.concatenate([res.results[c]["out"] for c in range(NCORES)],
                          axis=0)


# revision 24
# speedup vs baseline: 1.5667x; 1.1792x over previous
"""Trainium2 Bass kernel for nn_Attention_b (tanh-attention with masked_scatter).

Data-parallel over batch: each of 8 NeuronCores owns 4 batches. Per core:
  sweep A  z = W1 @ h_i + (W2 @ h_t + b)   (fp16 GEMM, [A, rows])
           m = tanh(z); y = u . m          (raw scores, [rows])
           AllGather of score pairs across the 8 cores (pipelined)
           then per pair: masked_scatter selection (0/1 matrix against
           gathered scores), chunk-local max + exp + running sums
  sweep B  chunk weights wn = exp(m_i - M)/L; scatter e*wn onto
           s-partitions as a block-diagonal [4b x 32s, 4] stationary
           (rank-1 PE matmuls); s[b, :] = sum_s e~ * h_iT streams on the
           PE with batch-interleaved moving data, PSUM-accumulated.
h_i is sent twice (fp16): once h-major for the GEMM, once s-major
batch-interleaved (row p = 32*b + j) for the weighted sum, so nothing
transposes the big tensor on chip and the weighted sum is one long
full-width PE stream.
"""
import sys

for _p in ("/opt/trn_rl_repo",):
    if _p not in sys.path:
        sys.path.insert(0, _p)

import numpy as np

import concourse.bacc as bacc
import concourse.tile as tile
from concourse import mybir
from concourse.bass_utils import run_bass_kernel_spmd

NCORES = 8
B, S, H, A = 32, 2048, 1024, 256
BL = B // NCORES          # local batches per core
NEG = np.float32(-1e20)

f32 = mybir.dt.float32
f32r = mybir.dt.float32r
f16 = mybir.dt.float16

SG = 32                   # seq positions per interleave group


def build_kernel(S=S, H=H, A=A, C=256, hi_bufs=2, hit_bufs=4):
    KT = H // 128             # contraction tiles
    AT = A // 128             # score tiles
    NCH = S // C              # chunks (phase-1/2 granularity)
    NPR = NCH // 2            # AllGather pairs
    NSG = S // SG             # interleave groups (32 seq x 4 batch each)
    SGC = C // SG             # groups per chunk
    assert H % 128 == 0 and A % 128 == 0 and S % (2 * C) == 0

    nc = bacc.Bacc("TRN2", target_bir_lowering=False, debug=False,
                   num_devices=NCORES)

    hi5 = nc.declare_dram_parameter("hi5", [128, KT * BL * S], f16,
                                    isOutput=False)
    hit = nc.declare_dram_parameter("hit", [128, NSG * H], f16,
                                    isOutput=False)
    w1t = nc.declare_dram_parameter("w1t", [H, A], f16, isOutput=False)
    cb2 = nc.declare_dram_parameter("cb2", [128, AT, BL], f32, isOutput=False)
    u2 = nc.declare_dram_parameter("u2", [128, AT], f16, isOutput=False)
    sel = nc.declare_dram_parameter("sel", [B + 1, BL, S], f16,
                                    isOutput=False)
    out = nc.declare_dram_parameter("out", [BL, H], f32, isOutput=True)

    with tile.TileContext(nc) as tc:
        with (
            tc.tile_pool(name="consts", bufs=1) as cp,
            tc.tile_pool(name="hi", bufs=hi_bufs) as hip,
            tc.tile_pool(name="hit", bufs=hit_bufs) as htp,
            tc.tile_pool(name="m", bufs=2) as mp,
            tc.tile_pool(name="small", bufs=3) as sp,
            tc.tile_pool(name="ps", bufs=3, space="PSUM") as pp,
            tc.tile_pool(name="dram", bufs=2 * NPR, space="DRAM") as dp,
        ):
            # ---- preload replicated constants
            w1_sb = cp.tile([128, KT, A], f16)
            nc.sync.dma_start(
                out=w1_sb, in_=w1t.rearrange("(t p) a -> p t a", p=128))
            u_sb = cp.tile([128, AT], f16)
            nc.sync.dma_start(out=u_sb, in_=u2[:, :])
            cb_sb = cp.tile([128, AT, BL], f32)
            nc.sync.dma_start(out=cb_sb, in_=cb2[:, :, :])
            ones_sb = cp.tile([B + 1, 1], f32r)
            nc.vector.memset(ones_sb.bitcast(f32), 1.0)

            # ---- per-chunk softmax stats + resident exp values
            mall = cp.tile([1, BL, NCH], f32)
            lall = cp.tile([1, BL, NCH], f32)
            e_all = cp.tile([1, BL, S], f16)
            eTi = cp.tile([128, NSG, BL], f16)

            def phase1(i):
                off = C * i
                hi_sb = hip.tile([128, KT, BL, C], f16, tag="hi")
                nc.sync.dma_start(
                    out=hi_sb.rearrange("p t b s -> p (t b s)"),
                    in_=hi5[:, KT * BL * off : KT * BL * (off + C)])
                sel_c = sp.tile([B + 1, BL, C], f16, tag="selc", bufs=NCH)
                nc.scalar.dma_start(out=sel_c, in_=sel[:, :, off : off + C])
                m_r = mp.tile([128, AT, BL, C], f16, tag="m")
                for at in range(AT):
                    z_ps = pp.tile([128, BL, C], f32, tag="zy")
                    for r in range(BL // 2):
                        for kt in range(KT):
                            nc.tensor.matmul(
                                z_ps[:, 2 * r : 2 * r + 2, :],
                                w1_sb[:, kt, at * 128 : (at + 1) * 128],
                                hi_sb[:, kt, 2 * r : 2 * r + 2, :],
                                start=(kt == 0), stop=(kt == KT - 1),
                            )
                    for b in range(BL):
                        nc.scalar.activation(
                            out=m_r[:, at, b, :], in_=z_ps[:, b, :],
                            func=mybir.ActivationFunctionType.Tanh,
                            bias=cb_sb[:, at, b : b + 1], scale=1.0,
                        )
                y_ps = pp.tile([1, BL, C], f32, tag="zy")
                for r in range(BL // 2):
                    for at in range(AT):
                        nc.tensor.matmul(
                            y_ps[:, 2 * r : 2 * r + 2, :],
                            u_sb[:, at : at + 1],
                            m_r[:, at, 2 * r : 2 * r + 2, :],
                            start=(at == 0), stop=(at == AT - 1),
                        )
                return dict(y_ps=y_ps, sel_c=sel_c, i=i)

            def aggather(c0, c1):
                """Copy the two chunks' scores out and AllGather the pair."""
                y_sb = sp.tile([1, BL, 2, C], f32, tag="ysb", bufs=1)
                nc.scalar.activation(out=y_sb[:, :, 0, :], in_=c0["y_ps"],
                                     func=mybir.ActivationFunctionType.Copy)
                nc.scalar.activation(out=y_sb[:, :, 1, :], in_=c1["y_ps"],
                                     func=mybir.ActivationFunctionType.Copy)
                ag_in = dp.tile([2 * BL * C], f32, tag="agin")
                nc.scalar.dma_start(
                    out=ag_in.rearrange("(o n) -> o n", o=1),
                    in_=y_sb.rearrange("p b c s -> p (b c s)"))
                ag_out = dp.tile([2 * B * C], f32, tag="agout",
                                 addr_space="Shared")
                nc.gpsimd.collective_compute(
                    "AllGather", mybir.AluOpType.bypass,
                    ins=[ag_in[:]], outs=[ag_out[:]],
                    replica_groups=[list(range(NCORES))],
                )
                # gathered rows: [(core, b), (chunk-of-pair, s)]
                y32 = sp.tile([B + 1, 2, C], f32, tag="y32", bufs=4)
                nc.gpsimd.memset(y32[B : B + 1, :, :], 1.0)
                nc.gpsimd.dma_start(
                    out=y32[:B].rearrange("j c s -> j (c s)"),
                    in_=ag_out.rearrange("(j n) -> j n", n=2 * C))
                return dict(y32=y32, h0=c0, h1=c1)

            def phase2(c, y32):
                i, sel_c = c["i"], c["sel_c"]
                # masked_scatter selection: one-hot rows (plus a -1e20 mask
                # row) dotted with [y; 1]
                bt_ps = pp.tile([1, BL, C], f32, tag="zy")
                selY = sp.tile([B + 1, BL, C], f32r, tag="selY", bufs=2)
                nc.vector.tensor_mul(
                    selY, sel_c,
                    y32.rearrange("j (o s) -> j o s", o=1)
                       .broadcast_to([B + 1, BL, C]))
                for hf in range(2):
                    nc.tensor.matmul(
                        bt_ps[:, 2 * hf : 2 * hf + 2, :], ones_sb,
                        selY[:, 2 * hf : 2 * hf + 2, :],
                        start=True, stop=True)

                # chunk-local max -> no cross-chunk recurrence
                cmax = sp.tile([1, BL], f32, tag="cmax")
                nc.vector.tensor_reduce(
                    out=cmax.rearrange("p (b o) -> p b o", o=1), in_=bt_ps,
                    axis=mybir.AxisListType.X, op=mybir.AluOpType.max)
                nc.vector.tensor_copy(mall[:, :, i], cmax)
                nmnew = sp.tile([1, BL], f32, tag="nmnew")
                nc.vector.tensor_scalar_mul(nmnew, cmax, -1.0)
                for b in range(BL):
                    nc.scalar.activation(
                        out=e_all[:, b, C * i : C * (i + 1)],
                        in_=bt_ps[:, b, :],
                        func=mybir.ActivationFunctionType.Exp,
                        bias=nmnew[:, b : b + 1], scale=1.0,
                        accum_out=lall[:, b, i : i + 1])

            # ---- sweep A: scores + pipelined AllGathers (no phase2 yet, so
            # the Act queue never blocks on a collective)
            pairs = []
            for p in range(NPR):
                c0 = phase1(2 * p)
                c1 = phase1(2 * p + 1)
                pairs.append(aggather(c0, c1))

            # ---- stream the s-major batch-interleaved copy of h_i
            hit_tiles = []
            for j in range(NCH):
                ht = htp.tile([128, SGC, H], f16, tag="hit")
                nc.sync.dma_start(
                    out=ht.rearrange("p c h -> p (c h)"),
                    in_=hit[:, SGC * H * j : SGC * H * (j + 1)])
                hit_tiles.append(ht)

            # ---- phase 2 for all pairs (selection + chunk softmax stats)
            for ag in pairs:
                phase2(ag["h0"], ag["y32"][:, 0, :])
                phase2(ag["h1"], ag["y32"][:, 1, :])

            # ---- combine chunk stats: wn[b, i] = exp(m_i - M) / L_b
            M = sp.tile([1, BL], f32, tag="cmax")
            nc.vector.tensor_reduce(
                out=M.rearrange("p (b o) -> p b o", o=1), in_=mall,
                axis=mybir.AxisListType.X, op=mybir.AluOpType.max)
            nM = sp.tile([1, BL], f32, tag="nmnew")
            nc.vector.tensor_scalar_mul(nM, M, -1.0)
            w = sp.tile([1, BL, NCH], f32, tag="w")
            for b in range(BL):
                nc.scalar.activation(
                    out=w[:, b, :], in_=mall[:, b, :],
                    func=mybir.ActivationFunctionType.Exp,
                    bias=nM[:, b : b + 1], scale=1.0)
            wl = sp.tile([1, BL, NCH], f32, tag="wl")
            nc.vector.tensor_mul(wl, w, lall)
            lsum = sp.tile([1, BL], f32, tag="lsum")
            nc.vector.tensor_reduce(
                out=lsum.rearrange("p (b o) -> p b o", o=1), in_=wl,
                axis=mybir.AxisListType.X, op=mybir.AluOpType.add)
            il = sp.tile([1, BL], f32, tag="il")
            nc.vector.reciprocal(il, lsum)
            wn = sp.tile([1, BL, NCH], f32, tag="wn")
            for b in range(BL):
                nc.vector.tensor_scalar_mul(wn[:, b, :], w[:, b, :],
                                            il[:, b : b + 1])
            wn16 = sp.tile([1, BL, NCH], f16, tag="wn16")
            nc.vector.tensor_copy(wn16, wn)

            # ---- eTi[32b+j, g, b] = e_all[b, g*32+j] * wn[b, chunk(g)]
            # (block-diagonal scatter onto s-partitions via rank-1 matmuls)
            eTi_ps = pp.tile([128, NSG, BL], f32, tag="zy")
            nc.vector.memset(eTi_ps, 0.0)
            for g in range(NSG):
                for b in range(BL):
                    nc.tensor.matmul(
                        eTi_ps[32 * b : 32 * b + 32, g, b : b + 1],
                        e_all[:, b, SG * g : SG * (g + 1)],
                        wn16[:, b, g // SGC : g // SGC + 1],
                        start=True, stop=True,
                        tile_position=(0, 32 * b),
                    )
            nc.scalar.activation(out=eTi, in_=eTi_ps,
                                 func=mybir.ActivationFunctionType.Copy)

            # ---- sweep B: s[b, :] += eTi.T @ hiT, one PSUM accumulation
            # (all-start=False + explicit zero: a start=True would mark the
            # whole 2KB PSUM zero region pending and wipe the other
            # interleaved groups' contributions)
            p3 = pp.tile([BL, H], f32, tag="zy")
            nc.vector.memset(p3, 0.0)
            HH = H // 512
            for j in range(NCH):
                ht = hit_tiles[j]
                for c in range(SGC):
                    g = SGC * j + c
                    for hh in range(HH):
                        nc.tensor.matmul(
                            p3[:, 512 * hh : 512 * (hh + 1)],
                            eTi[:, g, :],
                            ht[:, c, 512 * hh : 512 * (hh + 1)],
                            start=False, stop=(g == NSG - 1),
                            skip_group_check=True,
                        )
            out_sb = sp.tile([BL, H], f32, tag="osb")
            nc.scalar.activation(out=out_sb, in_=p3,
                                 func=mybir.ActivationFunctionType.Copy)
            nc.sync.dma_start(out=out[:, :], in_=out_sb)

    nc.compile()
    _split_pe_waits(nc)
    return nc


def _split_pe_waits(nc):
    """TRN2 PE instructions (S3_LW encoding) take a single sync-wait slot.
    Bacc's legalization misses some Matmults; hoist excess waits onto
    dedicated PE NoOps inserted directly before the offender."""
    for f in nc.m.functions:
        for bb in f.blocks:
            insts = bb.instructions
            i = 0
            while i < len(insts):
                ins = insts[i]
                if type(ins).__name__ in ("InstMatmult", "InstNoOp") and \
                        ins.engine == mybir.EngineType.PE:
                    si = ins.sync_info
                    if si is not None and len(si.on_wait) > 1:
                        extra, keep = si.on_wait[:-1], si.on_wait[-1:]
                        for w in extra:
                            nop = mybir.InstNoOp(
                                name=nc.get_next_instruction_name(),
                                ins=[], outs=[])
                            nop.engine = ins.engine
                            nop.sync_info = mybir.SyncInfo(
                                on_wait=[w], on_update=[])
                            nc.register_instruction(nop)
                            insts.insert(i, nop)
                            i += 1
                        si.on_wait = keep
                i += 1


def prep_inputs(h_i, h_t, mask, W, b, u, S=S, H=H, A=A, C=256):
    """Shard + lay out the full inputs for the 8 cores."""
    h_i = np.asarray(h_i, np.float32)
    h_t = np.asarray(h_t, np.float32)
    mask = np.asarray(mask, bool)
    W = np.asarray(W, np.float32)
    b = np.asarray(b, np.float32)
    u = np.asarray(u, np.float32)

    KT = H // 128
    AT = A // 128
    NSG = S // SG
    w1t = np.ascontiguousarray(W[:, :H].T).astype(np.float16)   # [H, A]
    cb = h_t @ W[:, H:].T + b                                   # [B, A]
    cb2s = np.ascontiguousarray(
        cb.reshape(B, AT, 128).transpose(2, 1, 0))              # [128, AT, B]
    u2 = np.ascontiguousarray(
        u[:, 0].reshape(AT, 128).T).astype(np.float16)          # [128, AT]

    pos = np.clip(np.cumsum(mask.astype(np.int64), axis=0) - 1, 0, None)
    onehot = (np.arange(B)[None, :, None] == pos[:, None, :]) & mask[:, None, :]
    selall = onehot.astype(np.float32)                          # [B, B, S]
    negall = np.where(mask, np.float32(0), np.float32(-60000.0))
    sel33 = np.concatenate([selall, negall[:, None, :]],
                           axis=1).astype(np.float16)  # [B, B+1, S]

    in_maps = []
    for c in range(NCORES):
        bs = slice(c * BL, (c + 1) * BL)
        hcf = h_i[bs].astype(np.float16)                    # [BL, S, H]
        # hi5[p, chunk_i ++ (t, b, s)] = h_i[b, off_i+s, t*128+p]
        h4 = hcf.reshape(BL, S // C, C, KT, 128)
        hi5 = np.ascontiguousarray(
            h4.transpose(4, 1, 3, 0, 2).reshape(128, KT * BL * S))
        # hit[32*b+j, (g, h)] = h_i[b, g*32+j, h]  (batch-interleaved rows)
        h5 = hcf.reshape(BL, NSG, SG, H).transpose(0, 2, 1, 3)  # [b, j, g, h]
        hitm = np.ascontiguousarray(h5.reshape(128, NSG * H))
        in_maps.append({
            "hi5": hi5,
            "hit": hitm,
            "w1t": w1t,
            "cb2": np.ascontiguousarray(cb2s[:, :, bs]),
            "u2": u2,
            "sel": np.ascontiguousarray(sel33[bs].transpose(1, 0, 2)),
        })
    return in_maps


_NC_CACHE = {}


def _get_nc():
    if "nc" not in _NC_CACHE:
        _NC_CACHE["nc"] = build_kernel()
    return _NC_CACHE["nc"]


def kernel(h_i, h_t, mask, W, b, u):
    nc = _get_nc()
    in_maps = prep_inputs(h_i, h_t, mask, W, b, u)
    res = run_bass_kernel_spmd(nc, in_maps, list(range(NCORES)))
    return np.concatenate([res.results[c]["out"] for c in range(NCORES)],
                          axis=0)


# revision 26
# speedup vs baseline: 1.7144x; 1.0943x over previous
"""Trainium2 Bass kernel for nn_Attention_b (tanh-attention with masked_scatter).

Data-parallel over batch: each of 8 NeuronCores owns 4 batches. Per core:
  sweep A  z = W1 @ h_i + (W2 @ h_t + b)   (fp16 GEMM, [A, rows])
           m = tanh(z); y = u . m          (raw scores, [rows])
           AllGather of score pairs across the 8 cores (pipelined)
           then per pair: masked_scatter selection (0/1 matrix against
           gathered scores), chunk-local max + exp + running sums
  sweep B  chunk weights wn = exp(m_i - M)/L; scatter e*wn onto
           s-partitions via rank-1 PE matmuls (transpose for free);
           s[b, :] = sum_s e~ * h_iT streams on the PE (e columns
           stationary, per-b output rows at PSUM partitions 32*b).
h_i is sent twice (fp16): once h-major for the GEMM, once s-major for
the weighted sum, so nothing transposes the big tensor on chip.
"""
import sys

for _p in ("/opt/trn_rl_repo",):
    if _p not in sys.path:
        sys.path.insert(0, _p)

import numpy as np

import concourse.bacc as bacc
import concourse.tile as tile
from concourse import mybir
from concourse.bass_utils import run_bass_kernel_spmd

NCORES = 8
B, S, H, A = 32, 2048, 1024, 256
BL = B // NCORES          # local batches per core
NEG = np.float32(-1e20)

f32 = mybir.dt.float32
f32r = mybir.dt.float32r
f16 = mybir.dt.float16

def build_kernel(S=S, H=H, A=A, C=256, hi_bufs=2, hit_bufs=4):
    KT = H // 128             # contraction tiles
    AT = A // 128             # score tiles
    NCH = S // C              # chunks (phase-1/2 granularity)
    NPR = NCH // 2            # AllGather pairs
    NSB = S // 128            # seq 128-blocks (phase-3 granularity)
    SBC = C // 128            # seq blocks per chunk
    assert H % 128 == 0 and A % 128 == 0 and S % (2 * C) == 0

    nc = bacc.Bacc("TRN2", target_bir_lowering=False, debug=False,
                   num_devices=NCORES)

    hi5 = nc.declare_dram_parameter("hi5", [128, KT * BL * S], f16,
                                    isOutput=False)
    hit = nc.declare_dram_parameter("hit", [128, NSB * BL * H], f16,
                                    isOutput=False)
    w1t = nc.declare_dram_parameter("w1t", [H, A], f16, isOutput=False)
    cb2 = nc.declare_dram_parameter("cb2", [128, AT, BL], f32, isOutput=False)
    u2 = nc.declare_dram_parameter("u2", [128, AT], f16, isOutput=False)
    sel = nc.declare_dram_parameter("sel", [B + 1, BL, S], f16,
                                    isOutput=False)
    out = nc.declare_dram_parameter("out", [BL, H], f32, isOutput=True)

    with tile.TileContext(nc) as tc:
        with (
            tc.tile_pool(name="consts", bufs=1) as cp,
            tc.tile_pool(name="hi", bufs=hi_bufs) as hip,
            tc.tile_pool(name="hit", bufs=hit_bufs) as htp,
            tc.tile_pool(name="m", bufs=2) as mp,
            tc.tile_pool(name="small", bufs=3) as sp,
            tc.tile_pool(name="ps", bufs=4, space="PSUM") as pp,
            tc.tile_pool(name="dram", bufs=2 * NPR, space="DRAM") as dp,
        ):
            # ---- preload replicated constants
            w1_sb = cp.tile([128, KT, A], f16)
            nc.sync.dma_start(
                out=w1_sb, in_=w1t.rearrange("(t p) a -> p t a", p=128))
            u_sb = cp.tile([128, AT], f16)
            nc.sync.dma_start(out=u_sb, in_=u2[:, :])
            cb_sb = cp.tile([128, AT, BL], f32)
            nc.sync.dma_start(out=cb_sb, in_=cb2[:, :, :])
            ones_sb = cp.tile([B + 1, 1], f32r)
            nc.vector.memset(ones_sb.bitcast(f32), 1.0)

            # ---- per-chunk softmax stats + resident exp values
            mall = cp.tile([1, BL, NCH], f32)
            lall = cp.tile([1, BL, NCH], f32)
            e_all = cp.tile([1, BL, S], f16)
            eT = cp.tile([128, NSB, BL], f16)

            def phase1(i):
                off = C * i
                hi_sb = hip.tile([128, KT, BL, C], f16, tag="hi")
                nc.sync.dma_start(
                    out=hi_sb.rearrange("p t b s -> p (t b s)"),
                    in_=hi5[:, KT * BL * off : KT * BL * (off + C)])
                sel_c = sp.tile([B + 1, BL, C], f16, tag="selc", bufs=NCH)
                nc.scalar.dma_start(out=sel_c, in_=sel[:, :, off : off + C])
                m_r = mp.tile([128, AT, BL, C], f16, tag="m")
                for at in range(AT):
                    z_ps = pp.tile([128, BL, C], f32, tag="zy")
                    for r in range(BL // 2):
                        for kt in range(KT):
                            nc.tensor.matmul(
                                z_ps[:, 2 * r : 2 * r + 2, :],
                                w1_sb[:, kt, at * 128 : (at + 1) * 128],
                                hi_sb[:, kt, 2 * r : 2 * r + 2, :],
                                start=(kt == 0), stop=(kt == KT - 1),
                            )
                    for b in range(BL):
                        nc.scalar.activation(
                            out=m_r[:, at, b, :], in_=z_ps[:, b, :],
                            func=mybir.ActivationFunctionType.Tanh,
                            bias=cb_sb[:, at, b : b + 1], scale=1.0,
                        )
                return dict(m_r=m_r, sel_c=sel_c, i=i)

            def phase1y(c):
                m_r = c.pop("m_r")
                y_ps = pp.tile([1, BL, C], f32, tag="zy")
                for r in range(BL // 2):
                    for at in range(AT):
                        nc.tensor.matmul(
                            y_ps[:, 2 * r : 2 * r + 2, :],
                            u_sb[:, at : at + 1],
                            m_r[:, at, 2 * r : 2 * r + 2, :],
                            start=(at == 0), stop=(at == AT - 1),
                        )
                c["y_ps"] = y_ps
                return c

            def aggather(c0, c1):
                """Copy the two chunks' scores out and AllGather the pair."""
                y_sb = sp.tile([1, BL, 2, C], f32, tag="ysb", bufs=1)
                nc.scalar.activation(out=y_sb[:, :, 0, :], in_=c0["y_ps"],
                                     func=mybir.ActivationFunctionType.Copy)
                nc.scalar.activation(out=y_sb[:, :, 1, :], in_=c1["y_ps"],
                                     func=mybir.ActivationFunctionType.Copy)
                ag_in = dp.tile([2 * BL * C], f32, tag="agin")
                nc.scalar.dma_start(
                    out=ag_in.rearrange("(o n) -> o n", o=1),
                    in_=y_sb.rearrange("p b c s -> p (b c s)"))
                ag_out = dp.tile([2 * B * C], f32, tag="agout",
                                 addr_space="Shared")
                nc.gpsimd.collective_compute(
                    "AllGather", mybir.AluOpType.bypass,
                    ins=[ag_in[:]], outs=[ag_out[:]],
                    replica_groups=[list(range(NCORES))],
                )
                return dict(ag_out=ag_out, h0=c0, h1=c1)

            def reload(ag):
                # gathered rows: [(core, b), (chunk-of-pair, s)]
                y32 = sp.tile([B + 1, 2, C], f32, tag="y32", bufs=4)
                nc.gpsimd.memset(y32[B : B + 1, :, :], 1.0)
                nc.gpsimd.dma_start(
                    out=y32[:B].rearrange("j c s -> j (c s)"),
                    in_=ag["ag_out"].rearrange("(j n) -> j n", n=2 * C))
                return y32

            def phase2(c, y32):
                i, sel_c = c["i"], c["sel_c"]
                # masked_scatter selection: one-hot rows (plus a -1e20 mask
                # row) dotted with [y; 1]
                bt_ps = pp.tile([1, BL, C], f32, tag="zy")
                selY = sp.tile([B + 1, BL, C], f32r, tag="selY", bufs=2)
                nc.vector.tensor_mul(
                    selY, sel_c,
                    y32.rearrange("j (o s) -> j o s", o=1)
                       .broadcast_to([B + 1, BL, C]))
                for hf in range(2):
                    nc.tensor.matmul(
                        bt_ps[:, 2 * hf : 2 * hf + 2, :], ones_sb,
                        selY[:, 2 * hf : 2 * hf + 2, :],
                        start=True, stop=True)

                # chunk-local max -> no cross-chunk recurrence
                cmax = sp.tile([1, BL], f32, tag="cmax")
                nc.vector.tensor_reduce(
                    out=cmax.rearrange("p (b o) -> p b o", o=1), in_=bt_ps,
                    axis=mybir.AxisListType.X, op=mybir.AluOpType.max)
                nc.vector.tensor_copy(mall[:, :, i], cmax)
                nmnew = sp.tile([1, BL], f32, tag="nmnew")
                nc.vector.tensor_scalar_mul(nmnew, cmax, -1.0)
                for b in range(BL):
                    nc.scalar.activation(
                        out=e_all[:, b, C * i : C * (i + 1)],
                        in_=bt_ps[:, b, :],
                        func=mybir.ActivationFunctionType.Exp,
                        bias=nmnew[:, b : b + 1], scale=1.0,
                        accum_out=lall[:, b, i : i + 1])

            # ---- sweep A: scores + pipelined AllGathers (no phase2 yet, so
            # the Act queue never blocks on a collective; each chunk's y
            # matmuls issue after the next chunk's z so the PE never stalls
            # on the tanh)
            pairs = []
            done, prev = [], None
            for i in range(NCH):
                cur = phase1(i)
                if prev is not None:
                    done.append(phase1y(prev))
                prev = cur
            done.append(phase1y(prev))
            for p in range(NPR):
                pairs.append(aggather(done[2 * p], done[2 * p + 1]))

            # ---- stream the s-major batch-interleaved copy of h_i
            hit_tiles = []
            for j in range(NCH):
                ht = htp.tile([128, SBC, BL, H], f16, tag="hit")
                nc.sync.dma_start(
                    out=ht.rearrange("p c b h -> p (c b h)"),
                    in_=hit[:, SBC * BL * H * j : SBC * BL * H * (j + 1)])
                hit_tiles.append(ht)

            # ---- phase 2 for all pairs (selection + chunk softmax stats)
            for ag in pairs:
                y32 = reload(ag)
                phase2(ag["h0"], y32[:, 0, :])
                phase2(ag["h1"], y32[:, 1, :])

            # ---- combine chunk stats: wn[b, i] = exp(m_i - M) / L_b
            M = sp.tile([1, BL], f32, tag="cmax")
            nc.vector.tensor_reduce(
                out=M.rearrange("p (b o) -> p b o", o=1), in_=mall,
                axis=mybir.AxisListType.X, op=mybir.AluOpType.max)
            nM = sp.tile([1, BL], f32, tag="nmnew")
            nc.vector.tensor_scalar_mul(nM, M, -1.0)
            w = sp.tile([1, BL, NCH], f32, tag="w")
            for b in range(BL):
                nc.scalar.activation(
                    out=w[:, b, :], in_=mall[:, b, :],
                    func=mybir.ActivationFunctionType.Exp,
                    bias=nM[:, b : b + 1], scale=1.0)
            wl = sp.tile([1, BL, NCH], f32, tag="wl")
            nc.vector.tensor_mul(wl, w, lall)
            lsum = sp.tile([1, BL], f32, tag="lsum")
            nc.vector.tensor_reduce(
                out=lsum.rearrange("p (b o) -> p b o", o=1), in_=wl,
                axis=mybir.AxisListType.X, op=mybir.AluOpType.add)
            il = sp.tile([1, BL], f32, tag="il")
            nc.vector.reciprocal(il, lsum)
            wn = sp.tile([1, BL, NCH], f32, tag="wn")
            for b in range(BL):
                nc.vector.tensor_scalar_mul(wn[:, b, :], w[:, b, :],
                                            il[:, b : b + 1])
            wn16 = sp.tile([1, BL, NCH], f16, tag="wn16")
            nc.vector.tensor_copy(wn16, wn)

            # ---- eT[j, sb, b] = e_all[b, sb*128+j] * wn[b, chunk(sb)]
            # (scatter onto s-partitions via rank-1 matmuls)
            eT_ps = pp.tile([128, NSB, BL], f32, tag="zy")
            for sb in range(NSB):
                for b in range(BL):
                    nc.tensor.matmul(
                        eT_ps[:, sb, b : b + 1],
                        e_all[:, b, 128 * sb : 128 * (sb + 1)],
                        wn16[:, b, sb // SBC : sb // SBC + 1],
                        start=True, stop=True,
                    )
            nc.scalar.activation(out=eT, in_=eT_ps,
                                 func=mybir.ActivationFunctionType.Copy)

            # ---- sweep B: s[b, :] += eTi.T @ hiT, one PSUM accumulation
            # (all-start=False + explicit zero: a start=True would mark the
            # whole 2KB PSUM zero region pending and wipe the other
            # interleaved groups' contributions)
            p3 = pp.tile([128, H], f32, tag="zy")
            nc.vector.memset(p3, 0.0)
            HH = H // 512
            for j in range(NCH):
                ht = hit_tiles[j]
                for c in range(SBC):
                    sb = SBC * j + c
                    for b in range(BL):
                        for hh in range(HH):
                            nc.tensor.matmul(
                                p3[32 * b : 32 * b + 1,
                                   512 * hh : 512 * (hh + 1)],
                                eT[:, sb, b : b + 1],
                                ht[:, c, b, 512 * hh : 512 * (hh + 1)],
                                start=False, stop=(sb == NSB - 1),
                                skip_group_check=True,
                                tile_position=(0, 32 * b),
                            )
            out_sb = sp.tile([128, H], f32, tag="osb")
            nc.scalar.activation(out=out_sb, in_=p3,
                                 func=mybir.ActivationFunctionType.Copy)
            nc.sync.dma_start(
                out=out[:, :],
                in_=out_sb.rearrange("(b r) h -> b r h", r=32)[:, 0, :])

    nc.compile()
    _split_pe_waits(nc)
    return nc


def _split_pe_waits(nc):
    """TRN2 PE instructions (S3_LW encoding) take a single sync-wait slot.
    Bacc's legalization misses some Matmults; hoist excess waits onto
    dedicated PE NoOps inserted directly before the offender."""
    for f in nc.m.functions:
        for bb in f.blocks:
            insts = bb.instructions
            i = 0
            while i < len(insts):
                ins = insts[i]
                if type(ins).__name__ in ("InstMatmult", "InstNoOp") and \
                        ins.engine == mybir.EngineType.PE:
                    si = ins.sync_info
                    if si is not None and len(si.on_wait) > 1:
                        extra, keep = si.on_wait[:-1], si.on_wait[-1:]
                        for w in extra:
                            nop = mybir.InstNoOp(
                                name=nc.get_next_instruction_name(),
                                ins=[], outs=[])
                            nop.engine = ins.engine
                            nop.sync_info = mybir.SyncInfo(
                                on_wait=[w], on_update=[])
                            nc.register_instruction(nop)
                            insts.insert(i, nop)
                            i += 1
                        si.on_wait = keep
                i += 1


def prep_inputs(h_i, h_t, mask, W, b, u, S=S, H=H, A=A, C=256):
    """Shard + lay out the full inputs for the 8 cores."""
    h_i = np.asarray(h_i, np.float32)
    h_t = np.asarray(h_t, np.float32)
    mask = np.asarray(mask, bool)
    W = np.asarray(W, np.float32)
    b = np.asarray(b, np.float32)
    u = np.asarray(u, np.float32)

    KT = H // 128
    AT = A // 128
    NSB = S // 128
    w1t = np.ascontiguousarray(W[:, :H].T).astype(np.float16)   # [H, A]
    cb = h_t @ W[:, H:].T + b                                   # [B, A]
    cb2s = np.ascontiguousarray(
        cb.reshape(B, AT, 128).transpose(2, 1, 0))              # [128, AT, B]
    u2 = np.ascontiguousarray(
        u[:, 0].reshape(AT, 128).T).astype(np.float16)          # [128, AT]

    pos = np.clip(np.cumsum(mask.astype(np.int64), axis=0) - 1, 0, None)
    onehot = (np.arange(B)[None, :, None] == pos[:, None, :]) & mask[:, None, :]
    selall = onehot.astype(np.float32)                          # [B, B, S]
    negall = np.where(mask, np.float32(0), np.float32(-60000.0))
    sel33 = np.concatenate([selall, negall[:, None, :]],
                           axis=1).astype(np.float16)  # [B, B+1, S]

    in_maps = []
    for c in range(NCORES):
        bs = slice(c * BL, (c + 1) * BL)
        hcf = h_i[bs].astype(np.float16)                    # [BL, S, H]
        # hi5[p, chunk_i ++ (t, b, s)] = h_i[b, off_i+s, t*128+p]
        h4 = hcf.reshape(BL, S // C, C, KT, 128)
        hi5 = np.ascontiguousarray(
            h4.transpose(4, 1, 3, 0, 2).reshape(128, KT * BL * S))
        # hit[p, (sb, b, h)] = h_i[b, sb*128+p, h]
        h5 = hcf.reshape(BL, NSB, 128, H)
        hitm = np.ascontiguousarray(
            h5.transpose(2, 1, 0, 3).reshape(128, NSB * BL * H))
        in_maps.append({
            "hi5": hi5,
            "hit": hitm,
            "w1t": w1t,
            "cb2": np.ascontiguousarray(cb2s[:, :, bs]),
            "u2": u2,
            "sel": np.ascontiguousarray(sel33[bs].transpose(1, 0, 2)),
        })
    return in_maps


_NC_CACHE = {}


def _get_nc():
    if "nc" not in _NC_CACHE:
        _NC_CACHE["nc"] = build_kernel()
    return _NC_CACHE["nc"]


def kernel(h_i, h_t, mask, W, b, u):
    nc = _get_nc()
    in_maps = prep_inputs(h_i, h_t, mask, W, b, u)
    res = run_bass_kernel_spmd(nc, in_maps, list(range(NCORES)))
    return np.concatenate([res.results[c]["out"] for c in range(NCORES)],
                          axis=0)


# revision 28
# speedup vs baseline: 1.7342x; 1.0116x over previous
"""Trainium2 Bass kernel for nn_Attention_b (tanh-attention with masked_scatter).

Data-parallel over batch: each of 8 NeuronCores owns 4 batches. Per core:
  sweep A  z = W1 @ h_i + (W2 @ h_t + b)   (fp16 GEMM, [A, rows])
           m = tanh(z); y = u . m          (raw scores, [rows])
           AllGather of score pairs across the 8 cores (pipelined)
           then per pair: masked_scatter selection (0/1 matrix against
           gathered scores), chunk-local max + exp + running sums
  sweep B  chunk weights wn = exp(m_i - M)/L; scatter e*wn onto
           s-partitions via rank-1 PE matmuls (transpose for free);
           s[b, :] = sum_s e~ * h_iT streams on the PE (e columns
           stationary, per-b output rows at PSUM partitions 32*b).
h_i is sent twice (fp16): once h-major for the GEMM, once s-major for
the weighted sum, so nothing transposes the big tensor on chip.
"""
import sys

for _p in ("/opt/trn_rl_repo",):
    if _p not in sys.path:
        sys.path.insert(0, _p)

import numpy as np

import concourse.bacc as bacc
import concourse.tile as tile
from concourse import mybir
from concourse.bass_utils import run_bass_kernel_spmd

NCORES = 8
B, S, H, A = 32, 2048, 1024, 256
BL = B // NCORES          # local batches per core
NEG = np.float32(-1e20)

f32 = mybir.dt.float32
f32r = mybir.dt.float32r
f16 = mybir.dt.float16

def build_kernel(S=S, H=H, A=A, C=256, hi_bufs=2, hit_bufs=5):
    KT = H // 128             # contraction tiles
    AT = A // 128             # score tiles
    NCH = S // C              # chunks (phase-1/2 granularity)
    NPR = NCH // 2            # AllGather pairs
    NSB = S // 128            # seq 128-blocks (phase-3 granularity)
    SBC = C // 128            # seq blocks per chunk
    assert H % 128 == 0 and A % 128 == 0 and S % (2 * C) == 0

    nc = bacc.Bacc("TRN2", target_bir_lowering=False, debug=False,
                   num_devices=NCORES)

    hi5 = nc.declare_dram_parameter("hi5", [128, KT * BL * S], f16,
                                    isOutput=False)
    hit = nc.declare_dram_parameter("hit", [128, NSB * BL * H], f16,
                                    isOutput=False)
    w1t = nc.declare_dram_parameter("w1t", [H, A], f16, isOutput=False)
    cb2 = nc.declare_dram_parameter("cb2", [128, AT, BL], f32, isOutput=False)
    u2 = nc.declare_dram_parameter("u2", [128, AT], f16, isOutput=False)
    sel = nc.declare_dram_parameter("sel", [B + 1, BL, S], f16,
                                    isOutput=False)
    out = nc.declare_dram_parameter("out", [BL, H], f32, isOutput=True)

    with tile.TileContext(nc) as tc:
        with (
            tc.tile_pool(name="consts", bufs=1) as cp,
            tc.tile_pool(name="hi", bufs=hi_bufs) as hip,
            tc.tile_pool(name="hit", bufs=hit_bufs) as htp,
            tc.tile_pool(name="m", bufs=2) as mp,
            tc.tile_pool(name="small", bufs=3) as sp,
            tc.tile_pool(name="ps", bufs=4, space="PSUM") as pp,
            tc.tile_pool(name="dram", bufs=2 * NPR, space="DRAM") as dp,
        ):
            # ---- preload replicated constants
            w1_sb = cp.tile([128, KT, A], f16)
            nc.sync.dma_start(
                out=w1_sb, in_=w1t.rearrange("(t p) a -> p t a", p=128))
            u_sb = cp.tile([128, AT], f16)
            nc.sync.dma_start(out=u_sb, in_=u2[:, :])
            cb_sb = cp.tile([128, AT, BL], f32)
            nc.sync.dma_start(out=cb_sb, in_=cb2[:, :, :])
            ones_sb = cp.tile([B + 1, 1], f32r)
            nc.vector.memset(ones_sb.bitcast(f32), 1.0)

            # ---- per-chunk softmax stats + resident exp values
            mall = cp.tile([1, BL, NCH], f32)
            lall = cp.tile([1, BL, NCH], f32)
            e_all = cp.tile([1, BL, S], f16)
            eT = cp.tile([128, NSB, BL], f16)

            def phase1(i):
                off = C * i
                hi_sb = hip.tile([128, KT, BL, C], f16, tag="hi")
                nc.sync.dma_start(
                    out=hi_sb.rearrange("p t b s -> p (t b s)"),
                    in_=hi5[:, KT * BL * off : KT * BL * (off + C)])
                sel_c = sp.tile([B + 1, BL, C], f16, tag="selc", bufs=NCH)
                nc.scalar.dma_start(out=sel_c, in_=sel[:, :, off : off + C])
                m_r = mp.tile([128, AT, BL, C], f16, tag="m")
                for at in range(AT):
                    z_ps = pp.tile([128, BL, C], f32, tag="zy")
                    for r in range(BL // 2):
                        for kt in range(KT):
                            nc.tensor.matmul(
                                z_ps[:, 2 * r : 2 * r + 2, :],
                                w1_sb[:, kt, at * 128 : (at + 1) * 128],
                                hi_sb[:, kt, 2 * r : 2 * r + 2, :],
                                start=(kt == 0), stop=(kt == KT - 1),
                            )
                    for b in range(BL):
                        nc.scalar.activation(
                            out=m_r[:, at, b, :], in_=z_ps[:, b, :],
                            func=mybir.ActivationFunctionType.Tanh,
                            bias=cb_sb[:, at, b : b + 1], scale=1.0,
                        )
                return dict(m_r=m_r, sel_c=sel_c, i=i)

            def phase1y(c):
                m_r = c.pop("m_r")
                y_ps = pp.tile([1, BL, C], f32, tag="zy")
                for r in range(BL // 2):
                    for at in range(AT):
                        nc.tensor.matmul(
                            y_ps[:, 2 * r : 2 * r + 2, :],
                            u_sb[:, at : at + 1],
                            m_r[:, at, 2 * r : 2 * r + 2, :],
                            start=(at == 0), stop=(at == AT - 1),
                        )
                c["y_ps"] = y_ps
                return c

            def aggather(c0, c1):
                """Copy the two chunks' scores out and AllGather the pair."""
                y_sb = sp.tile([1, BL, 2, C], f32, tag="ysb", bufs=1)
                nc.scalar.activation(out=y_sb[:, :, 0, :], in_=c0["y_ps"],
                                     func=mybir.ActivationFunctionType.Copy)
                nc.scalar.activation(out=y_sb[:, :, 1, :], in_=c1["y_ps"],
                                     func=mybir.ActivationFunctionType.Copy)
                ag_in = dp.tile([2 * BL * C], f32, tag="agin")
                nc.scalar.dma_start(
                    out=ag_in.rearrange("(o n) -> o n", o=1),
                    in_=y_sb.rearrange("p b c s -> p (b c s)"))
                ag_out = dp.tile([2 * B * C], f32, tag="agout",
                                 addr_space="Shared")
                nc.gpsimd.collective_compute(
                    "AllGather", mybir.AluOpType.bypass,
                    ins=[ag_in[:]], outs=[ag_out[:]],
                    replica_groups=[list(range(NCORES))],
                )
                return dict(ag_out=ag_out, h0=c0, h1=c1)

            def reload(ag):
                # gathered rows: [(core, b), (chunk-of-pair, s)]
                y32 = sp.tile([B + 1, 2, C], f32, tag="y32", bufs=4)
                nc.gpsimd.memset(y32[B : B + 1, :, :], 1.0)
                nc.gpsimd.dma_start(
                    out=y32[:B].rearrange("j c s -> j (c s)"),
                    in_=ag["ag_out"].rearrange("(j n) -> j n", n=2 * C))
                return y32

            def phase2(c, y32):
                i, sel_c = c["i"], c["sel_c"]
                # masked_scatter selection: one-hot rows (plus a -1e20 mask
                # row) dotted with [y; 1]
                bt_ps = pp.tile([1, BL, C], f32, tag="zy")
                selY = sp.tile([B + 1, BL, C], f32r, tag="selY", bufs=2)
                nc.vector.tensor_mul(
                    selY, sel_c,
                    y32.rearrange("j (o s) -> j o s", o=1)
                       .broadcast_to([B + 1, BL, C]))
                for hf in range(2):
                    nc.tensor.matmul(
                        bt_ps[:, 2 * hf : 2 * hf + 2, :], ones_sb,
                        selY[:, 2 * hf : 2 * hf + 2, :],
                        start=True, stop=True)

                # chunk-local max -> no cross-chunk recurrence
                cmax = sp.tile([1, BL], f32, tag="cmax")
                nc.vector.tensor_reduce(
                    out=cmax.rearrange("p (b o) -> p b o", o=1), in_=bt_ps,
                    axis=mybir.AxisListType.X, op=mybir.AluOpType.max)
                nc.vector.tensor_copy(mall[:, :, i], cmax)
                nmnew = sp.tile([1, BL], f32, tag="nmnew")
                nc.vector.tensor_scalar_mul(nmnew, cmax, -1.0)
                for b in range(BL):
                    nc.scalar.activation(
                        out=e_all[:, b, C * i : C * (i + 1)],
                        in_=bt_ps[:, b, :],
                        func=mybir.ActivationFunctionType.Exp,
                        bias=nmnew[:, b : b + 1], scale=1.0,
                        accum_out=lall[:, b, i : i + 1])

            # ---- sweep A: scores + pipelined AllGathers (no phase2 yet, so
            # the Act queue never blocks on a collective; each chunk's y
            # matmuls issue after the next chunk's z so the PE never stalls
            # on the tanh)
            pairs = []
            done, prev = [], None
            for i in range(NCH):
                cur = phase1(i)
                if prev is not None:
                    done.append(phase1y(prev))
                    if len(done) % 2 == 0:
                        pairs.append(aggather(done[-2], done[-1]))
                prev = cur
            done.append(phase1y(prev))
            pairs.append(aggather(done[-2], done[-1]))

            # ---- stream the s-major batch-interleaved copy of h_i
            hit_tiles = []
            for j in range(NCH):
                ht = htp.tile([128, SBC, BL, H], f16, tag="hit")
                nc.sync.dma_start(
                    out=ht.rearrange("p c b h -> p (c b h)"),
                    in_=hit[:, SBC * BL * H * j : SBC * BL * H * (j + 1)])
                hit_tiles.append(ht)

            # ---- phase 2 for all pairs (selection + chunk softmax stats)
            for ag in pairs:
                y32 = reload(ag)
                phase2(ag["h0"], y32[:, 0, :])
                phase2(ag["h1"], y32[:, 1, :])

            # ---- combine chunk stats: wn[b, i] = exp(m_i - M) / L_b
            M = sp.tile([1, BL], f32, tag="cmax")
            nc.vector.tensor_reduce(
                out=M.rearrange("p (b o) -> p b o", o=1), in_=mall,
                axis=mybir.AxisListType.X, op=mybir.AluOpType.max)
            nM = sp.tile([1, BL], f32, tag="nmnew")
            nc.vector.tensor_scalar_mul(nM, M, -1.0)
            w = sp.tile([1, BL, NCH], f32, tag="w")
            for b in range(BL):
                nc.scalar.activation(
                    out=w[:, b, :], in_=mall[:, b, :],
                    func=mybir.ActivationFunctionType.Exp,
                    bias=nM[:, b : b + 1], scale=1.0)
            wl = sp.tile([1, BL, NCH], f32, tag="wl")
            nc.vector.tensor_mul(wl, w, lall)
            lsum = sp.tile([1, BL], f32, tag="lsum")
            nc.vector.tensor_reduce(
                out=lsum.rearrange("p (b o) -> p b o", o=1), in_=wl,
                axis=mybir.AxisListType.X, op=mybir.AluOpType.add)
            il = sp.tile([1, BL], f32, tag="il")
            nc.vector.reciprocal(il, lsum)
            wn = sp.tile([1, BL, NCH], f32, tag="wn")
            for b in range(BL):
                nc.vector.tensor_scalar_mul(wn[:, b, :], w[:, b, :],
                                            il[:, b : b + 1])
            wn16 = sp.tile([1, BL, NCH], f16, tag="wn16")
            nc.vector.tensor_copy(wn16, wn)

            # ---- eT[j, sb, b] = e_all[b, sb*128+j] * wn[b, chunk(sb)]
            # (scatter onto s-partitions via rank-1 matmuls)
            eT_ps = pp.tile([128, NSB, BL], f32, tag="zy")
            for sb in range(NSB):
                for b in range(BL):
                    nc.tensor.matmul(
                        eT_ps[:, sb, b : b + 1],
                        e_all[:, b, 128 * sb : 128 * (sb + 1)],
                        wn16[:, b, sb // SBC : sb // SBC + 1],
                        start=True, stop=True,
                    )
            nc.scalar.activation(out=eT, in_=eT_ps,
                                 func=mybir.ActivationFunctionType.Copy)

            # ---- sweep B: s[b, :] += eTi.T @ hiT, one PSUM accumulation
            # (all-start=False + explicit zero: a start=True would mark the
            # whole 2KB PSUM zero region pending and wipe the other
            # interleaved groups' contributions)
            p3 = pp.tile([128, H], f32, tag="zy")
            nc.vector.memset(p3, 0.0)
            HH = H // 512
            for j in range(NCH):
                ht = hit_tiles[j]
                for c in range(SBC):
                    sb = SBC * j + c
                    for b in range(BL):
                        for hh in range(HH):
                            nc.tensor.matmul(
                                p3[32 * b : 32 * b + 1,
                                   512 * hh : 512 * (hh + 1)],
                                eT[:, sb, b : b + 1],
                                ht[:, c, b, 512 * hh : 512 * (hh + 1)],
                                start=False, stop=(sb == NSB - 1),
                                skip_group_check=True,
                                tile_position=(0, 32 * b),
                            )
            out_sb = sp.tile([128, H], f32, tag="osb")
            nc.scalar.activation(out=out_sb, in_=p3,
                                 func=mybir.ActivationFunctionType.Copy)
            nc.sync.dma_start(
                out=out[:, :],
                in_=out_sb.rearrange("(b r) h -> b r h", r=32)[:, 0, :])

    nc.compile()
    _split_pe_waits(nc)
    return nc


def _split_pe_waits(nc):
    """TRN2 PE instructions (S3_LW encoding) take a single sync-wait slot.
    Bacc's legalization misses some Matmults; hoist excess waits onto
    dedicated PE NoOps inserted directly before the offender."""
    for f in nc.m.functions:
        for bb in f.blocks:
            insts = bb.instructions
            i = 0
            while i < len(insts):
                ins = insts[i]
                if type(ins).__name__ in ("InstMatmult", "InstNoOp") and \
                        ins.engine == mybir.EngineType.PE:
                    si = ins.sync_info
                    if si is not None and len(si.on_wait) > 1:
                        extra, keep = si.on_wait[:-1], si.on_wait[-1:]
                        for w in extra:
                            nop = mybir.InstNoOp(
                                name=nc.get_next_instruction_name(),
                                ins=[], outs=[])
                            nop.engine = ins.engine
                            nop.sync_info = mybir.SyncInfo(
                                on_wait=[w], on_update=[])
                            nc.register_instruction(nop)
                            insts.insert(i, nop)
                            i += 1
                        si.on_wait = keep
                i += 1


def prep_inputs(h_i, h_t, mask, W, b, u, S=S, H=H, A=A, C=256):
    """Shard + lay out the full inputs for the 8 cores."""
    h_i = np.asarray(h_i, np.float32)
    h_t = np.asarray(h_t, np.float32)
    mask = np.asarray(mask, bool)
    W = np.asarray(W, np.float32)
    b = np.asarray(b, np.float32)
    u = np.asarray(u, np.float32)

    KT = H // 128
    AT = A // 128
    NSB = S // 128
    w1t = np.ascontiguousarray(W[:, :H].T).astype(np.float16)   # [H, A]
    cb = h_t @ W[:, H:].T + b                                   # [B, A]
    cb2s = np.ascontiguousarray(
        cb.reshape(B, AT, 128).transpose(2, 1, 0))              # [128, AT, B]
    u2 = np.ascontiguousarray(
        u[:, 0].reshape(AT, 128).T).astype(np.float16)          # [128, AT]

    pos = np.clip(np.cumsum(mask.astype(np.int64), axis=0) - 1, 0, None)
    onehot = (np.arange(B)[None, :, None] == pos[:, None, :]) & mask[:, None, :]
    selall = onehot.astype(np.float32)                          # [B, B, S]
    negall = np.where(mask, np.float32(0), np.float32(-60000.0))
    sel33 = np.concatenate([selall, negall[:, None, :]],
                           axis=1).astype(np.float16)  # [B, B+1, S]

    in_maps = []
    for c in range(NCORES):
        bs = slice(c * BL, (c + 1) * BL)
        hcf = h_i[bs].astype(np.float16)                    # [BL, S, H]
        # hi5[p, chunk_i ++ (t, b, s)] = h_i[b, off_i+s, t*128+p]
        h4 = hcf.reshape(BL, S // C, C, KT, 128)
        hi5 = np.ascontiguousarray(
            h4.transpose(4, 1, 3, 0, 2).reshape(128, KT * BL * S))
        # hit[p, (sb, b, h)] = h_i[b, sb*128+p, h]
        h5 = hcf.reshape(BL, NSB, 128, H)
        hitm = np.ascontiguousarray(
            h5.transpose(2, 1, 0, 3).reshape(128, NSB * BL * H))
        in_maps.append({
            "hi5": hi5,
            "hit": hitm,
            "w1t": w1t,
            "cb2": np.ascontiguousarray(cb2s[:, :, bs]),
            "u2": u2,
            "sel": np.ascontiguousarray(sel33[bs].transpose(1, 0, 2)),
        })
    return in_maps


_NC_CACHE = {}


def _get_nc():
    if "nc" not in _NC_CACHE:
        _NC_CACHE["nc"] = build_kernel()
    return _NC_CACHE["nc"]


def kernel(h_i, h_t, mask, W, b, u):
    nc = _get_nc()
    in_maps = prep_inputs(h_i, h_t, mask, W, b, u)
    res = run_bass_kernel_spmd(nc, in_maps, list(range(NCORES)))
    return np.concatenate([res.results[c]["out"] for c in range(NCORES)],
                          axis=0)
